# revision 22
# baseline (speedup 1.0000x reference)
"""Trainium2 Bass kernel for nn_MultiHeadAttention (B=4, S=2048, D=512, H=8, dk=dv=64).

Sharding: core c = 2*b + j handles batch b, q-rows [1024*j, 1024*(j+1)), all 8 heads.
No cross-core communication needed; host gathers per-core outputs.

Returns (x, attn) matching reference.reference(**inputs).
"""
import os
import sys

sys.path.insert(0, "/opt/trn_rl_repo")

KBISECT = os.environ.get("KBISECT", "full")

import numpy as np

import concourse.bass as bass  # noqa: F401
import concourse.tile as tile
from concourse import bacc, mybir
from concourse import bass_utils
from concourse.masks import make_identity

F32 = mybir.dt.float32
F32R = mybir.dt.float32r
BF16 = mybir.dt.bfloat16
AF = mybir.ActivationFunctionType
ALU = mybir.AluOpType

N_CORES = 8
B, S, D = 4, 2048, 512
H, DK, DV = 8, 64, 64
SQ = S // 2          # 1024 q rows per core
LN_EPS = 1e-5


def _patch_act_tables():
    """Force Exp+Ln onto the combined natural_log_exp_and_others set so the
    ACT table is loaded once instead of ping-ponging per head."""
    import concourse.bacc as _bacc_mod
    from concourse.hw_specs import get_activation_tables as _orig

    def patched(arch):
        tables = _orig(arch)
        both = {AF.Exp, AF.Ln}
        for name, fns in tables.items():
            if name != "natural_log_exp_and_others" and (fns & both):
                tables[name] = fns - both
        return tables

    _bacc_mod.get_activation_tables = patched


def _build_nc():
    _patch_act_tables()
    nc = bacc.Bacc("TRN2", target_bir_lowering=False, debug=False,
                   enable_asserts=True, num_devices=N_CORES)

    qT_d = nc.dram_tensor("qT", [D, SQ], F32R, kind="ExternalInput").ap()
    kT_d = nc.dram_tensor("kT", [D, S], F32R, kind="ExternalInput").ap()
    vT_d = nc.dram_tensor("vT", [D, S], F32R, kind="ExternalInput").ap()
    qres_d = nc.dram_tensor("qres", [SQ, D], F32, kind="ExternalInput").ap()
    wqsT_d = nc.dram_tensor("wqsT", [D, D], F32R, kind="ExternalInput").ap()
    wksT_d = nc.dram_tensor("wksT", [D, D], F32R, kind="ExternalInput").ap()
    wvsT_d = nc.dram_tensor("wvsT", [D, D], F32R, kind="ExternalInput").ap()
    wfcT_d = nc.dram_tensor("wfcT", [D, D], F32R, kind="ExternalInput").ap()
    bqs_d = nc.dram_tensor("bqs", [4, 128], F32, kind="ExternalInput").ap()
    bks_d = nc.dram_tensor("bks", [4, 128], F32, kind="ExternalInput").ap()
    bvs_d = nc.dram_tensor("bvs", [1, D], F32, kind="ExternalInput").ap()
    lng_d = nc.dram_tensor("lng", [1, D], F32, kind="ExternalInput").ap()
    lnb_d = nc.dram_tensor("lnb", [1, D], F32, kind="ExternalInput").ap()

    attn_d = nc.dram_tensor("attn_p", [H, SQ, S], F32, kind="ExternalOutput").ap()
    x_d = nc.dram_tensor("x_p", [SQ, D], F32, kind="ExternalOutput").ap()
    oc_dbg_d = (nc.dram_tensor("oc_dbg", [4, 128, SQ], F32,
                               kind="ExternalOutput").ap()
                if KBISECT == "dbg" else None)

    with tile.TileContext(nc) as tc:
        _emit(tc, nc, qT_d, kT_d, vT_d, qres_d, wqsT_d, wksT_d, wvsT_d, wfcT_d,
              bqs_d, bks_d, bvs_d, lng_d, lnb_d, attn_d, x_d, oc_dbg_d)
    nc.compile()
    return nc


def _emit(tc, nc, qT_d, kT_d, vT_d, qres_d, wqsT_d, wksT_d, wvsT_d, wfcT_d,
          bqs_d, bks_d, bvs_d, lng_d, lnb_d, attn_d, x_d, oc_dbg_d=None):
    # ---------------- persistent pools ----------------
    with tc.tile_pool(name="persist", bufs=1) as persist, \
         tc.tile_pool(name="consts", bufs=1) as consts:
        qhT = persist.tile([128, 4, SQ], F32R, tag="qhT")       # [dk*H, sq]
        khT = persist.tile([128, 4, S], F32R, tag="khT")        # [dk*H, s]
        vh_t = persist.tile([128, 16, 512], BF16, tag="vh")  # [s, h*v]
        ocT = persist.tile([128, 4, SQ], F32R, tag="ocT")       # [f, sq] normalized heads
        wfcT = persist.tile([128, 4, D], F32R, tag="wfcT")

        bqs_t = consts.tile([128, 4], F32, tag="bqs")
        bks_t = consts.tile([128, 4], F32, tag="bks")
        bvs_rep = consts.tile([128, D], F32, tag="bvsr")
        lng_rep = consts.tile([128, D], F32, tag="lngr")
        lnb_rep = consts.tile([128, D], F32, tag="lnbr")
        ones_row = consts.tile([65, 128], F32, tag="onesrow")
        ident = consts.tile([64, 64], F32, tag="ident")
        ident128 = consts.tile([128, 128], F32, tag="ident128")
        nc.vector.memset(ones_row[:], 1.0)
        make_identity(nc, ident128[:])

        eps_t = consts.tile([128, 1], F32, tag="epst")
        nc.vector.memset(eps_t[:], LN_EPS)
        make_identity(nc, ident[:])

        for ft in range(4):
            nc.sync.dma_start(wfcT[:, ft, :], wfcT_d.rearrange("(t p) f -> p t f", p=128)[:, ft, :])
            nc.sync.dma_start(bqs_t[:, ft:ft + 1], bqs_d[ft:ft + 1, :].rearrange("o p -> p o"))
            nc.sync.dma_start(bks_t[:, ft:ft + 1], bks_d[ft:ft + 1, :].rearrange("o p -> p o"))
        if KBISECT == "nopb":
            nc.vector.memset(bvs_rep[:], 0.0)
            nc.vector.memset(lng_rep[:], 1.0)
            nc.vector.memset(lnb_rep[:], 0.0)
        else:
            tmp_row = consts.tile([1, D], F32, tag="tmprow")
            nc.sync.dma_start(tmp_row[:], bvs_d)
            nc.gpsimd.partition_broadcast(bvs_rep[:], tmp_row[:])
            tmp_row2 = consts.tile([1, D], F32, tag="tmprow2")
            nc.sync.dma_start(tmp_row2[:], lng_d)
            nc.gpsimd.partition_broadcast(lng_rep[:], tmp_row2[:])
            tmp_row3 = consts.tile([1, D], F32, tag="tmprow3")
            nc.sync.dma_start(tmp_row3[:], lnb_d)
            nc.gpsimd.partition_broadcast(lnb_rep[:], tmp_row3[:])

        # ---------------- phase 1: projections ----------------
        with tc.tile_pool(name="win", bufs=1) as win, \
             tc.tile_pool(name="inp", bufs=1) as inp, \
             tc.tile_pool(name="pps", bufs=4, space="PSUM") as pps:
            wqsT = win.tile([128, 4, D], F32R, tag="wqsT")
            wksT = win.tile([128, 4, D], F32R, tag="wksT")
            wvsT = win.tile([128, 4, D], F32R, tag="wvsT")
            qT_t = inp.tile([128, 4, SQ], F32R, tag="qT")
            for dt in range(4):
                nc.sync.dma_start(wqsT[:, dt, :], wqsT_d.rearrange("(t p) f -> p t f", p=128)[:, dt, :])
                nc.sync.dma_start(wksT[:, dt, :], wksT_d.rearrange("(t p) f -> p t f", p=128)[:, dt, :])
                nc.sync.dma_start(wvsT[:, dt, :], wvsT_d.rearrange("(t p) f -> p t f", p=128)[:, dt, :])
                nc.sync.dma_start(qT_t[:, dt, :], qT_d.rearrange("(t p) s -> p t s", p=128)[:, dt, :])

            # qh^T [f, sq]
            for ft in range(4):
                for sc in range(SQ // 512):
                    pp = pps.tile([128, 512], F32, tag="pp")
                    for dt in range(4):
                        nc.tensor.matmul(
                            pp[:],
                            wqsT[:, dt, ft * 128:(ft + 1) * 128],
                            qT_t[:, dt, sc * 512:(sc + 1) * 512],
                            start=(dt == 0), stop=(dt == 3))
                    nc.vector.tensor_scalar_add(
                        qhT[:, ft, sc * 512:(sc + 1) * 512], pp[:],
                        bqs_t[:, ft:ft + 1])
            # kh^T [f, s] in s-halves (SBUF pressure)
            for half in range(2):
                s0 = half * 1024
                kT_h = inp.tile([128, 4, 1024], F32R, tag="kTh")
                for dt in range(4):
                    nc.sync.dma_start(
                        kT_h[:, dt, :],
                        kT_d.rearrange("(t p) s -> p t s", p=128)[:, dt, s0:s0 + 1024])
                for ft in range(4):
                    for sc in range(2):
                        pp = pps.tile([128, 512], F32, tag="pp")
                        for dt in range(4):
                            nc.tensor.matmul(
                                pp[:],
                                wksT[:, dt, ft * 128:(ft + 1) * 128],
                                kT_h[:, dt, sc * 512:(sc + 1) * 512],
                                start=(dt == 0), stop=(dt == 3))
                        nc.vector.tensor_scalar_add(
                            khT[:, ft, s0 + sc * 512:s0 + (sc + 1) * 512], pp[:],
                            bks_t[:, ft:ft + 1])
            # vh rows [s, f] into vaug strided (col 65h+64 = ones)
            for half in range(2):
                s0 = half * 1024
                vT_h = inp.tile([128, 4, 1024], F32R, tag="vTh")
                for dt in range(4):
                    nc.sync.dma_start(
                        vT_h[:, dt, :],
                        vT_d.rearrange("(t p) s -> p t s", p=128)[:, dt, s0:s0 + 1024])
                for st in range(8):
                    pp = pps.tile([128, 512], F32, tag="pp")
                    for dt in range(4):
                        nc.tensor.matmul(
                            pp[:],
                            vT_h[:, dt, st * 128:(st + 1) * 128],
                            wvsT[:, dt, :],
                            start=(dt == 0), stop=(dt == 3))
                    nc.vector.tensor_tensor(
                        vh_t[:, half * 8 + st, :], pp[:], bvs_rep[:], ALU.add)

        # ---------------- phase 2: attention, SW-pipelined ----------------
        with tc.tile_pool(name="expp", bufs=3) as expp, \
             tc.tile_pool(name="esp", bufs=4) as esp, \
             tc.tile_pool(name="outr", bufs=2) as outr, \
             tc.tile_pool(name="small", bufs=2) as small, \
             tc.tile_pool(name="st_ps", bufs=1, space="PSUM") as st_ps, \
             tc.tile_pool(name="s_ps", bufs=2, space="PSUM") as s_ps, \
             tc.tile_pool(name="av_ps", bufs=1, space="PSUM") as av_ps:

            def stage_a(item, state):
                hp, qc = item
                q0 = qc * 512
                avp = [av_ps.tile([64, 512], F32, tag=f"avp{r}", name=f"avp{r}")
                       for r in range(2)]
                for kt in range(16):
                    stp = st_ps.tile([128, 1024], F32, tag="stp", name="stp")
                    for r in range(2):
                        hr = r * 64
                        nc.tensor.matmul(
                            stp[:, r * 512:(r + 1) * 512],
                            khT[hr:hr + 64, hp, kt * 128:(kt + 1) * 128],
                            qhT[hr:hr + 64, hp, q0:q0 + 512],
                            start=True, stop=True)
                    expb = expp.tile([128, 2, 512], BF16, tag="expb", name="expb")
                    nc.scalar.activation(
                        expb[:], stp[:].rearrange("p (a b) -> p a b", a=2), AF.Exp)
                    for r in range(2):
                        h = 2 * hp + r
                        nc.tensor.matmul(
                            avp[r][:],
                            vh_t[:, kt, 64 * h:64 * h + 64],
                            expb[:, r, :],
                            start=(kt == 0), stop=(kt == 15))
                    yield
                orr = [outr.tile([64, 512], F32, tag=f"or{r}", name=f"or{r}")
                       for r in range(2)]
                for r in range(2):
                    nc.vector.tensor_copy(orr[r][:], avp[r][:])
                state["orr"] = orr

            def stage_b(item, state):
                hp, qc = item
                q0 = qc * 512
                inv_t = [small.tile([128, 4], F32, tag=f"inv{r}", name=f"inv{r}")
                         for r in range(2)]
                for i in range(4):
                    for r in range(2):
                        hr, h = r * 64, 2 * hp + r
                        sums2 = small.tile([128, 2], F32, tag="sums2")
                        es_pair = []
                        for half in range(2):
                            sp = s_ps.tile([128, 1024], F32, tag="sp", name="sp")
                            for t in range(2):
                                kc = half * 2 + t
                                nc.tensor.matmul(
                                    sp[:, t * 512:(t + 1) * 512],
                                    qhT[hr:hr + 64, hp,
                                        q0 + i * 128:q0 + (i + 1) * 128],
                                    khT[hr:hr + 64, hp, kc * 512:(kc + 1) * 512],
                                    start=True, stop=True)
                            es = esp.tile([128, 1024], F32, tag="es", name="es")
                            nc.scalar.activation(
                                es[:], sp[:], AF.Exp,
                                accum_out=sums2[:, half:half + 1])
                            es_pair.append(es)
                        sums = small.tile([128, 1], F32, tag="sums")
                        nc.vector.tensor_reduce(sums[:], sums2[:],
                                                mybir.AxisListType.X, ALU.add)
                        nc.vector.reciprocal(inv_t[r][:, i:i + 1], sums[:])
                        for half, es in enumerate(es_pair):
                            es2 = esp.tile([128, 1024], F32, tag="es2", name="es2")
                            nc.vector.tensor_scalar_mul(es2[:], es[:],
                                                        inv_t[r][:, i:i + 1])
                            nc.sync.dma_start(
                                attn_d[h, q0 + i * 128:q0 + (i + 1) * 128,
                                       half * 1024:(half + 1) * 1024],
                                es2[:])
                        yield
                # inv_row -> inv_rep -> normalize out^T into ocT
                for r in range(2):
                    trp = s_ps.tile([128, 512], F32, tag="sp", name="trp")
                    for i in range(4):
                        nc.tensor.transpose(
                            trp[0:1, i * 128:(i + 1) * 128],
                            inv_t[r][:, i:i + 1], ident128[:])
                    invrow = small.tile([1, 512], F32, tag="invrow")
                    nc.vector.tensor_copy(invrow[:], trp[0:1, :])
                    reprp = s_ps.tile([128, 512], F32, tag="sp", name="reprp")
                    nc.tensor.matmul(reprp[:], ones_row[0:1, :], invrow[:],
                                     start=True, stop=True)
                    inv_rep = small.tile([128, 512], F32, tag="invrep")
                    nc.vector.tensor_copy(inv_rep[:], reprp[:])
                    orr = state["orr"][r]
                    if r == 0:
                        nc.vector.tensor_tensor(
                            ocT[0:64, hp, q0:q0 + 512],
                            orr[:], inv_rep[0:64, :], ALU.mult)
                    else:
                        stage = small.tile([64, 512], F32, tag="stage")
                        nc.vector.tensor_tensor(
                            stage[:], orr[:], inv_rep[0:64, :], ALU.mult)
                        av2 = s_ps.tile([128, 512], F32, tag="sp", name="av2")
                        nc.tensor.matmul(av2[64:128, :], ident[:], stage[:],
                                         start=True, stop=True,
                                         tile_position=(0, 64))
                        nc.vector.tensor_copy(ocT[64:128, hp, q0:q0 + 512],
                                              av2[64:128, :])
                    yield

            def warm_burst(nmm=18):
                # dense, dependency-free matmuls to flip the PE HAM warm
                wp = st_ps.tile([128, 1024], F32, tag="stp", name="warm")
                for w in range(nmm):
                    nc.tensor.matmul(
                        wp[:, (w % 2) * 512:(w % 2) * 512 + 512],
                        vh_t[:, 0, 0:128], vh_t[:, 1, 0:512],
                        start=True, stop=True, skip_group_check=True)

            items = [(hp, qc) for hp in range(4) for qc in range(2)]
            if KBISECT == "nop2":
                items = []
            states = [dict() for _ in items]
            gens = []
            for n in range(len(items) + 1):
                if n % 2 == 0:
                    warm_burst()
                ga = iter(stage_a(items[n], states[n])) if n < len(items) else None
                gb = iter(stage_b(items[n - 1], states[n - 1])) if n >= 1 else None
                done = False
                while not done:
                    done = True
                    for g in (ga, gb):
                        if g is not None:
                            try:
                                next(g)
                                done = False
                            except StopIteration:
                                pass

        if oc_dbg_d is not None:
            for ft in range(4):
                nc.sync.dma_start(oc_dbg_d[ft, :, :],
                                  ocT[:, ft, :].bitcast(F32))
        # ---------------- phase 3: fc + residual + LN ----------------
        with tc.tile_pool(name="fc_ps", bufs=2, space="PSUM") as fc_ps, \
             tc.tile_pool(name="xp", bufs=2) as xp, \
             tc.tile_pool(name="stats", bufs=8) as stats:
            if KBISECT == "p1only":
                for st in range(8):
                    qr0 = xp.tile([128, D], F32, tag="qr")
                    nc.sync.dma_start(qr0[:], qres_d[st * 128:(st + 1) * 128, :])
                    nc.sync.dma_start(x_d[st * 128:(st + 1) * 128, :], qr0[:])
            for st in range(8 if KBISECT not in ("p1only",) else 0):
                fcp = fc_ps.tile([128, 512], F32, tag="fcp")
                for ft in range(4):
                    nc.tensor.matmul(
                        fcp[:],
                        (ocT if KBISECT != "nop2" else wfcT)[:, ft, 0:128] if KBISECT == "nop2" else ocT[:, ft, st * 128:(st + 1) * 128],
                        wfcT[:, ft, :],
                        start=(ft == 0), stop=(ft == 3))
                qr = xp.tile([128, D], F32, tag="qr")
                nc.sync.dma_start(qr[:], qres_d[st * 128:(st + 1) * 128, :])
                xt = xp.tile([128, D], F32, tag="xt")
                nc.vector.tensor_tensor(xt[:], fcp[:], qr[:], ALU.add)
                mu = stats.tile([128, 1], F32, tag="mu")
                nc.vector.tensor_reduce(mu[:], xt[:], mybir.AxisListType.X, ALU.add)
                mus = stats.tile([128, 1], F32, tag="mus")
                nc.vector.tensor_scalar_mul(mus[:], mu[:], 1.0 / D)
                xc = xp.tile([128, D], F32, tag="xc")
                nc.vector.tensor_scalar_sub(xc[:], xt[:], mus[:])
                sq_t = xp.tile([128, D], F32, tag="sq")
                vs = stats.tile([128, 1], F32, tag="vs")
                nc.vector.tensor_tensor(sq_t[:], xc[:], xc[:], ALU.mult)
                nc.vector.tensor_reduce(vs[:], sq_t[:], mybir.AxisListType.X, ALU.add)
                lnv = stats.tile([128, 1], F32, tag="lnv")
                nc.scalar.activation(lnv[:], vs[:], AF.Ln, scale=1.0 / D,
                                     bias=eps_t[:])
                istd = stats.tile([128, 1], F32, tag="istd")
                nc.scalar.activation(istd[:], lnv[:], AF.Exp, scale=-0.5)
                xn = xp.tile([128, D], F32, tag="xn")
                nc.vector.tensor_scalar_mul(xn[:], xc[:], istd[:])
                xg = xp.tile([128, D], F32, tag="xg")
                nc.vector.tensor_tensor(xg[:], xn[:], lng_rep[:], ALU.mult)
                xo = xp.tile([128, D], F32, tag="xo")
                nc.vector.tensor_tensor(xo[:], xg[:], lnb_rep[:], ALU.add)
                nc.sync.dma_start(x_d[st * 128:(st + 1) * 128, :], xo[:])


_NC_CACHE = None


def _get_nc():
    global _NC_CACHE
    if _NC_CACHE is None:
        _NC_CACHE = _build_nc()
    return _NC_CACHE


def kernel(q, k, v, w_qs, b_qs, w_ks, b_ks, w_vs, b_vs, w_fc, b_fc, ln_g, ln_b,
           _trace=False, _trace_kwargs=None):
    q = np.asarray(q, dtype=np.float32)
    k = np.asarray(k, dtype=np.float32)
    v = np.asarray(v, dtype=np.float32)
    temp = np.float32(np.power(DK, 0.5))
    wqsT = np.ascontiguousarray((np.asarray(w_qs, np.float32) / temp).T)
    wksT = np.ascontiguousarray(np.asarray(w_ks, np.float32).T)
    wvsT = np.ascontiguousarray(np.asarray(w_vs, np.float32).T)
    wfcT = np.ascontiguousarray(np.asarray(w_fc, np.float32).T)
    bqs = (np.asarray(b_qs, np.float32) / temp).reshape(4, 128)
    bks = np.asarray(b_ks, np.float32).reshape(4, 128)
    bvs = np.asarray(b_vs, np.float32).reshape(1, D)
    lng = np.asarray(ln_g, np.float32).reshape(1, D)
    lnb = np.asarray(ln_b, np.float32).reshape(1, D)
    bfc = np.asarray(b_fc, np.float32)

    in_maps = []
    for c in range(N_CORES):
        b, j = c // 2, c % 2
        qT = np.ascontiguousarray(q[b].T[:, j * SQ:(j + 1) * SQ])
        kT = np.ascontiguousarray(k[b].T)
        vT = np.ascontiguousarray(v[b].T)
        qres = np.ascontiguousarray(q[b, j * SQ:(j + 1) * SQ, :]) + bfc[None, :]
        in_maps.append({
            "qT": qT, "kT": kT, "vT": vT, "qres": qres,
            "wqsT": wqsT, "wksT": wksT, "wvsT": wvsT, "wfcT": wfcT,
            "bqs": bqs, "bks": bks, "bvs": bvs, "lng": lng, "lnb": lnb,
        })

    nc = _get_nc()
    kwargs = {}
    if _trace:
        kwargs["trace"] = True
        kwargs.update(_trace_kwargs or {})
    res = bass_utils.run_bass_kernel_spmd(
        nc, in_maps, core_ids=list(range(N_CORES)), **kwargs)

    x = np.empty((B, S, D), np.float32)
    attn = np.empty((H * B, S, S), np.float32)
    attn_v = attn.reshape(H, B, 2, SQ, S)
    for c in range(N_CORES):
        b, j = c // 2, c % 2
        x[b, j * SQ:(j + 1) * SQ, :] = res.results[c]["x_p"]
        attn_v[:, b, j, :, :] = res.results[c]["attn_p"]
    kernel._last_result = res
    return x, attn


# revision 23
# speedup vs baseline: 1.0962x; 1.0962x over previous
"""Trainium2 Bass kernel for nn_MultiHeadAttention (B=4, S=2048, D=512, H=8, dk=dv=64).

Sharding: core c = 2*b + j handles batch b, q-rows [1024*j, 1024*(j+1)), all 8 heads.
No cross-core communication needed; host gathers per-core outputs.

Returns (x, attn) matching reference.reference(**inputs).
"""
import os
import sys

sys.path.insert(0, "/opt/trn_rl_repo")

KBISECT = os.environ.get("KBISECT", "full")

import numpy as np

import concourse.bass as bass  # noqa: F401
import concourse.tile as tile
from concourse import bacc, mybir
from concourse import bass_utils
from concourse.masks import make_identity

F32 = mybir.dt.float32
F32R = mybir.dt.float32r
BF16 = mybir.dt.bfloat16
AF = mybir.ActivationFunctionType
ALU = mybir.AluOpType

N_CORES = 8
B, S, D = 4, 2048, 512
H, DK, DV = 8, 64, 64
SQ = S // 2          # 1024 q rows per core
LN_EPS = 1e-5


def _patch_act_tables():
    """Force Exp+Ln onto the combined natural_log_exp_and_others set so the
    ACT table is loaded once instead of ping-ponging per head."""
    import concourse.bacc as _bacc_mod
    from concourse.hw_specs import get_activation_tables as _orig

    def patched(arch):
        tables = _orig(arch)
        both = {AF.Exp, AF.Ln}
        for name, fns in tables.items():
            if name != "natural_log_exp_and_others" and (fns & both):
                tables[name] = fns - both
        return tables

    _bacc_mod.get_activation_tables = patched


def _build_nc():
    _patch_act_tables()
    nc = bacc.Bacc("TRN2", target_bir_lowering=False, debug=False,
                   enable_asserts=True, num_devices=N_CORES)

    qT_d = nc.dram_tensor("qT", [D, SQ], F32R, kind="ExternalInput").ap()
    kT_d = nc.dram_tensor("kT", [D, S], F32R, kind="ExternalInput").ap()
    vT_d = nc.dram_tensor("vT", [D, S], F32R, kind="ExternalInput").ap()
    qres_d = nc.dram_tensor("qres", [SQ, D], F32, kind="ExternalInput").ap()
    wqsT_d = nc.dram_tensor("wqsT", [D, D], F32R, kind="ExternalInput").ap()
    wksT_d = nc.dram_tensor("wksT", [D, D], F32R, kind="ExternalInput").ap()
    wvsT_d = nc.dram_tensor("wvsT", [D, D], F32R, kind="ExternalInput").ap()
    wfcT_d = nc.dram_tensor("wfcT", [D, D], F32R, kind="ExternalInput").ap()
    bqs_d = nc.dram_tensor("bqs", [4, 128], F32, kind="ExternalInput").ap()
    bks_d = nc.dram_tensor("bks", [4, 128], F32, kind="ExternalInput").ap()
    bvs_d = nc.dram_tensor("bvs", [1, D], F32, kind="ExternalInput").ap()
    lng_d = nc.dram_tensor("lng", [1, D], F32, kind="ExternalInput").ap()
    lnb_d = nc.dram_tensor("lnb", [1, D], F32, kind="ExternalInput").ap()

    attn_d = nc.dram_tensor("attn_p", [H, SQ, S], F32, kind="ExternalOutput").ap()
    x_d = nc.dram_tensor("x_p", [SQ, D], F32, kind="ExternalOutput").ap()
    oc_dbg_d = (nc.dram_tensor("oc_dbg", [4, 128, SQ], F32,
                               kind="ExternalOutput").ap()
                if KBISECT == "dbg" else None)

    with tile.TileContext(nc) as tc:
        _emit(tc, nc, qT_d, kT_d, vT_d, qres_d, wqsT_d, wksT_d, wvsT_d, wfcT_d,
              bqs_d, bks_d, bvs_d, lng_d, lnb_d, attn_d, x_d, oc_dbg_d)
    nc.compile()
    return nc


def _emit(tc, nc, qT_d, kT_d, vT_d, qres_d, wqsT_d, wksT_d, wvsT_d, wfcT_d,
          bqs_d, bks_d, bvs_d, lng_d, lnb_d, attn_d, x_d, oc_dbg_d=None):
    # ---------------- persistent pools ----------------
    with tc.tile_pool(name="persist", bufs=1) as persist, \
         tc.tile_pool(name="consts", bufs=1) as consts:
        qhT = persist.tile([128, 4, SQ], F32R, tag="qhT")       # [dk*H, sq]
        khT = persist.tile([128, 4, S], F32R, tag="khT")        # [dk*H, s]
        vh_t = persist.tile([128, 16, 512], BF16, tag="vh")  # [s, h*v]
        ocT = persist.tile([128, 4, SQ], F32R, tag="ocT")       # [f, sq] normalized heads
        wfcT = persist.tile([128, 4, D], F32R, tag="wfcT")

        bqs_t = consts.tile([128, 4], F32, tag="bqs")
        bks_t = consts.tile([128, 4], F32, tag="bks")
        bvs_rep = consts.tile([128, D], F32, tag="bvsr")
        lng_rep = consts.tile([128, D], F32, tag="lngr")
        lnb_rep = consts.tile([128, D], F32, tag="lnbr")
        ones_row = consts.tile([65, 128], F32, tag="onesrow")
        ident = consts.tile([64, 64], F32, tag="ident")
        ident128 = consts.tile([128, 128], F32, tag="ident128")
        nc.vector.memset(ones_row[:], 1.0)
        make_identity(nc, ident128[:])

        eps_t = consts.tile([128, 1], F32, tag="epst")
        nc.vector.memset(eps_t[:], LN_EPS)
        make_identity(nc, ident[:])

        for ft in range(4):
            nc.sync.dma_start(wfcT[:, ft, :], wfcT_d.rearrange("(t p) f -> p t f", p=128)[:, ft, :])
            nc.sync.dma_start(bqs_t[:, ft:ft + 1], bqs_d[ft:ft + 1, :].rearrange("o p -> p o"))
            nc.sync.dma_start(bks_t[:, ft:ft + 1], bks_d[ft:ft + 1, :].rearrange("o p -> p o"))
        if KBISECT == "nopb":
            nc.vector.memset(bvs_rep[:], 0.0)
            nc.vector.memset(lng_rep[:], 1.0)
            nc.vector.memset(lnb_rep[:], 0.0)
        else:
            tmp_row = consts.tile([1, D], F32, tag="tmprow")
            nc.sync.dma_start(tmp_row[:], bvs_d)
            nc.gpsimd.partition_broadcast(bvs_rep[:], tmp_row[:])
            tmp_row2 = consts.tile([1, D], F32, tag="tmprow2")
            nc.sync.dma_start(tmp_row2[:], lng_d)
            nc.gpsimd.partition_broadcast(lng_rep[:], tmp_row2[:])
            tmp_row3 = consts.tile([1, D], F32, tag="tmprow3")
            nc.sync.dma_start(tmp_row3[:], lnb_d)
            nc.gpsimd.partition_broadcast(lnb_rep[:], tmp_row3[:])

        # ---------------- phase 1: projections ----------------
        with tc.tile_pool(name="win", bufs=1) as win, \
             tc.tile_pool(name="inp", bufs=1) as inp, \
             tc.tile_pool(name="pps", bufs=4, space="PSUM") as pps:
            wqsT = win.tile([128, 4, D], F32R, tag="wqsT")
            wksT = win.tile([128, 4, D], F32R, tag="wksT")
            wvsT = win.tile([128, 4, D], F32R, tag="wvsT")
            qT_t = inp.tile([128, 4, SQ], F32R, tag="qT")
            for dt in range(4):
                nc.sync.dma_start(wqsT[:, dt, :], wqsT_d.rearrange("(t p) f -> p t f", p=128)[:, dt, :])
                nc.sync.dma_start(wksT[:, dt, :], wksT_d.rearrange("(t p) f -> p t f", p=128)[:, dt, :])
                nc.sync.dma_start(wvsT[:, dt, :], wvsT_d.rearrange("(t p) f -> p t f", p=128)[:, dt, :])
                nc.sync.dma_start(qT_t[:, dt, :], qT_d.rearrange("(t p) s -> p t s", p=128)[:, dt, :])

            # qh^T [f, sq]
            for ft in range(4):
                for sc in range(SQ // 512):
                    pp = pps.tile([128, 512], F32, tag="pp")
                    for dt in range(4):
                        nc.tensor.matmul(
                            pp[:],
                            wqsT[:, dt, ft * 128:(ft + 1) * 128],
                            qT_t[:, dt, sc * 512:(sc + 1) * 512],
                            start=(dt == 0), stop=(dt == 3))
                    nc.vector.tensor_scalar_add(
                        qhT[:, ft, sc * 512:(sc + 1) * 512], pp[:],
                        bqs_t[:, ft:ft + 1])
            # kh^T [f, s] in s-halves (SBUF pressure)
            for half in range(2):
                s0 = half * 1024
                kT_h = inp.tile([128, 4, 1024], F32R, tag="kTh")
                for dt in range(4):
                    nc.sync.dma_start(
                        kT_h[:, dt, :],
                        kT_d.rearrange("(t p) s -> p t s", p=128)[:, dt, s0:s0 + 1024])
                for ft in range(4):
                    for sc in range(2):
                        pp = pps.tile([128, 512], F32, tag="pp")
                        for dt in range(4):
                            nc.tensor.matmul(
                                pp[:],
                                wksT[:, dt, ft * 128:(ft + 1) * 128],
                                kT_h[:, dt, sc * 512:(sc + 1) * 512],
                                start=(dt == 0), stop=(dt == 3))
                        nc.vector.tensor_scalar_add(
                            khT[:, ft, s0 + sc * 512:s0 + (sc + 1) * 512], pp[:],
                            bks_t[:, ft:ft + 1])
            # vh rows [s, f] into vaug strided (col 65h+64 = ones)
            for half in range(2):
                s0 = half * 1024
                vT_h = inp.tile([128, 4, 1024], F32R, tag="vTh")
                for dt in range(4):
                    nc.sync.dma_start(
                        vT_h[:, dt, :],
                        vT_d.rearrange("(t p) s -> p t s", p=128)[:, dt, s0:s0 + 1024])
                for st in range(8):
                    pp = pps.tile([128, 512], F32, tag="pp")
                    for dt in range(4):
                        nc.tensor.matmul(
                            pp[:],
                            vT_h[:, dt, st * 128:(st + 1) * 128],
                            wvsT[:, dt, :],
                            start=(dt == 0), stop=(dt == 3))
                    nc.vector.tensor_tensor(
                        vh_t[:, half * 8 + st, :], pp[:], bvs_rep[:], ALU.add)

        # ---------------- phase 2: attention, SW-pipelined ----------------
        with tc.tile_pool(name="expp", bufs=3) as expp, \
             tc.tile_pool(name="esp", bufs=4) as esp, \
             tc.tile_pool(name="outr", bufs=2) as outr, \
             tc.tile_pool(name="small", bufs=2) as small, \
             tc.tile_pool(name="st_ps", bufs=2, space="PSUM") as st_ps, \
             tc.tile_pool(name="s_ps", bufs=1, space="PSUM") as s_ps, \
             tc.tile_pool(name="misc_ps", bufs=1, space="PSUM") as misc_ps, \
             tc.tile_pool(name="av_ps", bufs=1, space="PSUM") as av_ps:

            def stage_a(item, state):
                hp, qc = item
                q0 = qc * 512
                avp = av_ps.tile([128, 512], F32, tag="avp", name="avp")
                for kt in range(16):
                    stp = st_ps.tile([128, 1024], F32, tag="stp", name="stp")
                    for r in range(2):
                        hr = r * 64
                        nc.tensor.matmul(
                            stp[:, r * 512:(r + 1) * 512],
                            khT[hr:hr + 64, hp, kt * 128:(kt + 1) * 128],
                            qhT[hr:hr + 64, hp, q0:q0 + 512],
                            start=True, stop=True)
                    expb = expp.tile([128, 2, 512], BF16, tag="expb", name="expb")
                    nc.scalar.activation(
                        expb[:], stp[:].rearrange("p (a b) -> p a b", a=2), AF.Exp)
                    for r in range(2):
                        h = 2 * hp + r
                        nc.tensor.matmul(
                            avp[64 * r:64 * r + 64, :],
                            vh_t[:, kt, 64 * h:64 * h + 64],
                            expb[:, r, :],
                            start=(kt == 0), stop=(kt == 15),
                            tile_position=(0, 64 * r),
                            skip_group_check=True)
                    yield
                orr = outr.tile([128, 512], F32, tag="orr", name="orr")
                nc.vector.tensor_copy(orr[:], avp[:])
                state["orr"] = orr

            def stage_b(item, state):
                hp, qc = item
                q0 = qc * 512
                inv_t = [small.tile([128, 4], F32, tag=f"inv{r}", name=f"inv{r}")
                         for r in range(2)]
                for i in range(4):
                    for r in range(2):
                        hr, h = r * 64, 2 * hp + r
                        sums2 = small.tile([128, 2], F32, tag="sums2")
                        es_pair = []
                        for half in range(2):
                            sp = s_ps.tile([128, 1024], F32, tag="sp", name="sp")
                            for t in range(2):
                                kc = half * 2 + t
                                nc.tensor.matmul(
                                    sp[:, t * 512:(t + 1) * 512],
                                    qhT[hr:hr + 64, hp,
                                        q0 + i * 128:q0 + (i + 1) * 128],
                                    khT[hr:hr + 64, hp, kc * 512:(kc + 1) * 512],
                                    start=True, stop=True)
                            es = esp.tile([128, 1024], F32, tag="es", name="es")
                            nc.scalar.activation(
                                es[:], sp[:], AF.Exp,
                                accum_out=sums2[:, half:half + 1])
                            es_pair.append(es)
                        sums = small.tile([128, 1], F32, tag="sums")
                        nc.vector.tensor_reduce(sums[:], sums2[:],
                                                mybir.AxisListType.X, ALU.add)
                        nc.vector.reciprocal(inv_t[r][:, i:i + 1], sums[:])
                        for half, es in enumerate(es_pair):
                            es2 = esp.tile([128, 1024], F32, tag="es2", name="es2")
                            nc.vector.tensor_scalar_mul(es2[:], es[:],
                                                        inv_t[r][:, i:i + 1])
                            nc.sync.dma_start(
                                attn_d[h, q0 + i * 128:q0 + (i + 1) * 128,
                                       half * 1024:(half + 1) * 1024],
                                es2[:])
                        yield
                # inv_row -> inv_rep -> normalize out^T into ocT
                for r in range(2):
                    trp = misc_ps.tile([128, 512], F32, tag="misc", name="trp")
                    for i in range(4):
                        nc.tensor.transpose(
                            trp[0:1, i * 128:(i + 1) * 128],
                            inv_t[r][:, i:i + 1], ident128[:])
                    invrow = small.tile([1, 512], F32, tag="invrow")
                    nc.vector.tensor_copy(invrow[:], trp[0:1, :])
                    reprp = misc_ps.tile([128, 512], F32, tag="misc", name="reprp")
                    nc.tensor.matmul(reprp[:], ones_row[0:1, :], invrow[:],
                                     start=True, stop=True)
                    inv_rep = small.tile([128, 512], F32, tag="invrep")
                    nc.vector.tensor_copy(inv_rep[:], reprp[:])
                    orr = state["orr"]
                    p0 = 64 * r
                    nc.vector.tensor_tensor(
                        ocT[p0:p0 + 64, hp, q0:q0 + 512],
                        orr[p0:p0 + 64, :], inv_rep[p0:p0 + 64, :], ALU.mult)
                    yield

            def warm_burst(nmm=18):
                # dense, dependency-free matmuls to flip the PE HAM warm
                wp = misc_ps.tile([128, 512], F32, tag="misc", name="warm")
                for w in range(nmm):
                    nc.tensor.matmul(
                        wp[:], vh_t[:, 0, 0:128], vh_t[:, 1, 0:512],
                        start=True, stop=True, skip_group_check=True)

            items = [(hp, qc) for hp in range(4) for qc in range(2)]
            if KBISECT == "nop2":
                items = []
            states = [dict() for _ in items]
            gens = []
            for n in range(len(items) + 1):
                if n % 2 == 0:
                    warm_burst()
                ga = iter(stage_a(items[n], states[n])) if n < len(items) else None
                gb = iter(stage_b(items[n - 1], states[n - 1])) if n >= 1 else None
                done = False
                while not done:
                    done = True
                    for g in (ga, gb):
                        if g is not None:
                            try:
                                next(g)
                                done = False
                            except StopIteration:
                                pass

        if oc_dbg_d is not None:
            for ft in range(4):
                nc.sync.dma_start(oc_dbg_d[ft, :, :],
                                  ocT[:, ft, :].bitcast(F32))
        # ---------------- phase 3: fc + residual + LN ----------------
        with tc.tile_pool(name="fc_ps", bufs=2, space="PSUM") as fc_ps, \
             tc.tile_pool(name="xp", bufs=2) as xp, \
             tc.tile_pool(name="stats", bufs=8) as stats:
            if KBISECT == "p1only":
                for st in range(8):
                    qr0 = xp.tile([128, D], F32, tag="qr")
                    nc.sync.dma_start(qr0[:], qres_d[st * 128:(st + 1) * 128, :])
                    nc.sync.dma_start(x_d[st * 128:(st + 1) * 128, :], qr0[:])
            for st in range(8 if KBISECT not in ("p1only",) else 0):
                fcp = fc_ps.tile([128, 512], F32, tag="fcp")
                for ft in range(4):
                    nc.tensor.matmul(
                        fcp[:],
                        (ocT if KBISECT != "nop2" else wfcT)[:, ft, 0:128] if KBISECT == "nop2" else ocT[:, ft, st * 128:(st + 1) * 128],
                        wfcT[:, ft, :],
                        start=(ft == 0), stop=(ft == 3))
                qr = xp.tile([128, D], F32, tag="qr")
                nc.sync.dma_start(qr[:], qres_d[st * 128:(st + 1) * 128, :])
                xt = xp.tile([128, D], F32, tag="xt")
                nc.vector.tensor_tensor(xt[:], fcp[:], qr[:], ALU.add)
                mu = stats.tile([128, 1], F32, tag="mu")
                nc.vector.tensor_reduce(mu[:], xt[:], mybir.AxisListType.X, ALU.add)
                mus = stats.tile([128, 1], F32, tag="mus")
                nc.vector.tensor_scalar_mul(mus[:], mu[:], 1.0 / D)
                xc = xp.tile([128, D], F32, tag="xc")
                nc.vector.tensor_scalar_sub(xc[:], xt[:], mus[:])
                sq_t = xp.tile([128, D], F32, tag="sq")
                vs = stats.tile([128, 1], F32, tag="vs")
                nc.vector.tensor_tensor(sq_t[:], xc[:], xc[:], ALU.mult)
                nc.vector.tensor_reduce(vs[:], sq_t[:], mybir.AxisListType.X, ALU.add)
                lnv = stats.tile([128, 1], F32, tag="lnv")
                nc.scalar.activation(lnv[:], vs[:], AF.Ln, scale=1.0 / D,
                                     bias=eps_t[:])
                istd = stats.tile([128, 1], F32, tag="istd")
                nc.scalar.activation(istd[:], lnv[:], AF.Exp, scale=-0.5)
                xn = xp.tile([128, D], F32, tag="xn")
                nc.vector.tensor_scalar_mul(xn[:], xc[:], istd[:])
                xg = xp.tile([128, D], F32, tag="xg")
                nc.vector.tensor_tensor(xg[:], xn[:], lng_rep[:], ALU.mult)
                xo = xp.tile([128, D], F32, tag="xo")
                nc.vector.tensor_tensor(xo[:], xg[:], lnb_rep[:], ALU.add)
                nc.sync.dma_start(x_d[st * 128:(st + 1) * 128, :], xo[:])


_NC_CACHE = None


def _get_nc():
    global _NC_CACHE
    if _NC_CACHE is None:
        _NC_CACHE = _build_nc()
    return _NC_CACHE


def kernel(q, k, v, w_qs, b_qs, w_ks, b_ks, w_vs, b_vs, w_fc, b_fc, ln_g, ln_b,
           _trace=False, _trace_kwargs=None):
    q = np.asarray(q, dtype=np.float32)
    k = np.asarray(k, dtype=np.float32)
    v = np.asarray(v, dtype=np.float32)
    temp = np.float32(np.power(DK, 0.5))
    wqsT = np.ascontiguousarray((np.asarray(w_qs, np.float32) / temp).T)
    wksT = np.ascontiguousarray(np.asarray(w_ks, np.float32).T)
    wvsT = np.ascontiguousarray(np.asarray(w_vs, np.float32).T)
    wfcT = np.ascontiguousarray(np.asarray(w_fc, np.float32).T)
    bqs = (np.asarray(b_qs, np.float32) / temp).reshape(4, 128)
    bks = np.asarray(b_ks, np.float32).reshape(4, 128)
    bvs = np.asarray(b_vs, np.float32).reshape(1, D)
    lng = np.asarray(ln_g, np.float32).reshape(1, D)
    lnb = np.asarray(ln_b, np.float32).reshape(1, D)
    bfc = np.asarray(b_fc, np.float32)

    in_maps = []
    for c in range(N_CORES):
        b, j = c // 2, c % 2
        qT = np.ascontiguousarray(q[b].T[:, j * SQ:(j + 1) * SQ])
        kT = np.ascontiguousarray(k[b].T)
        vT = np.ascontiguousarray(v[b].T)
        qres = np.ascontiguousarray(q[b, j * SQ:(j + 1) * SQ, :]) + bfc[None, :]
        in_maps.append({
            "qT": qT, "kT": kT, "vT": vT, "qres": qres,
            "wqsT": wqsT, "wksT": wksT, "wvsT": wvsT, "wfcT": wfcT,
            "bqs": bqs, "bks": bks, "bvs": bvs, "lng": lng, "lnb": lnb,
        })

    nc = _get_nc()
    kwargs = {}
    if _trace:
        kwargs["trace"] = True
        kwargs.update(_trace_kwargs or {})
    res = bass_utils.run_bass_kernel_spmd(
        nc, in_maps, core_ids=list(range(N_CORES)), **kwargs)

    x = np.empty((B, S, D), np.float32)
    attn = np.empty((H * B, S, S), np.float32)
    attn_v = attn.reshape(H, B, 2, SQ, S)
    for c in range(N_CORES):
        b, j = c // 2, c % 2
        x[b, j * SQ:(j + 1) * SQ, :] = res.results[c]["x_p"]
        attn_v[:, b, j, :, :] = res.results[c]["attn_p"]
    kernel._last_result = res
    return x, attn
